# revision 12
# baseline (speedup 1.0000x reference)
"""Multi-head causal attention (B=4, T=2048, C=1024, H=16) on 8 TRN2 cores.

Sharding: core i handles batch b = i//2 and head-group g = i%2 (8 heads each).
Each core computes qkv projection for its heads, causal attention, and a
partial output projection (its heads' rows of W_o). The host sums the two
partials per batch and adds b_o.

Device kernel (per core, same SPMD program), all matmuls bf16 with fp32 PSUM,
fully software-pipelined by head pair so the qkv projection of head-pair
hp+1 fills the PE while the exp-paced attention of head-pair hp runs:

  - qkT = (Wqk^T x^T) transposed: [1024 feats, 2048] bf16, emitted per
    128-feature tile as pipeline filler
  - v   = x Wv natural: [2048, 512] bf16, per 128-token tile as filler
  - attention per head pair (2hp, 2hp+1), hp-major, per 512-query block tb,
    per 128-key chunk j (causal-trimmed):
      S^T = K^T q^T  [128 tk, tq]  (two heads at PE row groups 0-63/64-127,
                                    concurrent on HW), fp32 PSUM [128,1024]
      P^T = exp(S^T * 0.125)       (ScalarE; diagonal chunks masked on DVE)
      AV^T += [V | 1]^T P^T        [65, tq]  (row 64 = softmax denominator)
    S chunks of segment s+1 interleave with AV chunks of segment s.
    AV drains: even head on DVE, odd head on ACT (concurrent), sums on DVE.
  - softmax normalization per segment (lagged one segment): reciprocal on
    DVE, broadcast across partitions via ones-outer matmul into PSUM,
    single in-place DVE multiply of attn from the PSUM operand
  - out_part = attT^T W_o rows [2048, 1024]: per 128-token tile as stage-3
    filler as soon as its last segment normalizes; host sums the two
    per-batch partials in fp32 and adds b_o.

PSUM budget (8 banks): scores [128,1024]x2 = 4, AV [.,512]x2 = 2,
aux (proj/norm-broadcast/oproj) [128,512]x2 = 2.
"""

import sys

sys.path.insert(0, "/opt/trn_rl_repo")

import numpy as np
import ml_dtypes

BF16 = ml_dtypes.bfloat16

B, T, C, H, D = 4, 2048, 1024, 16, 64
HPC = 8        # heads per core
CQ = HPC * D   # 512
NCORES = 8
P = 128
NTT = T // 512  # 4 query blocks
VW = HPC * 65   # 520: v row layout (64 cols + ones col per head)


def _split_waits(nc):
    """This container's walrus accepts only ONE sync wait per instruction.
    Split any instruction carrying N>1 waits into N-1 single-wait NoOps on
    the same engine immediately before it."""
    import concourse.mybir as mybir

    ctr = 0
    for fn in nc.m.functions:
        for bb in fn.blocks:
            insts = list(bb.instructions)
            new_insts = []
            changed = False
            for inst in insts:
                si = inst.sync_info
                if si is not None and si.on_wait and len(si.on_wait) > 1:
                    waits = list(si.on_wait)
                    for w in waits[:-1]:
                        ctr += 1
                        nop = mybir.InstNoOp(
                            name=f"I-wsplit-{ctr}",
                            engine=inst.engine,
                            ins=[],
                            outs=[],
                            sync_info=mybir.SyncInfo(on_wait=[w], on_update=[]),
                        )
                        new_insts.append(nop)
                    si.on_wait = [waits[-1]]
                    changed = True
                new_insts.append(inst)
            if changed:
                bb.instructions[:] = new_insts
    return ctr


def _declare(nc):
    import concourse.mybir as mybir

    bf = mybir.dt.bfloat16
    f32 = mybir.dt.float32
    return dict(
        xT=nc.dram_tensor("xT", [C, T], bf, kind="ExternalInput").ap(),
        wqk=nc.dram_tensor("wqk", [C, 2 * CQ], bf, kind="ExternalInput").ap(),
        bqk=nc.dram_tensor("bqk", [P, 8], f32, kind="ExternalInput").ap(),
        wv=nc.dram_tensor("wv", [C, CQ], bf, kind="ExternalInput").ap(),
        bvb=nc.dram_tensor("bvb", [P, CQ], f32, kind="ExternalInput").ap(),
        wo=nc.dram_tensor("wo", [CQ, C], bf, kind="ExternalInput").ap(),
        maskT=nc.dram_tensor("maskT", [P, P], bf, kind="ExternalInput").ap(),
        outp=nc.dram_tensor("outp", [T, C], bf, kind="ExternalOutput").ap(),
    )


def _emit(nc, tc, aps):
    import concourse.mybir as mybir
    from concourse.alu_op_type import AluOpType

    bf = mybir.dt.bfloat16
    f32 = mybir.dt.float32
    Exp = mybir.ActivationFunctionType.Exp

    xT = aps["xT"]; wqk = aps["wqk"]; bqk = aps["bqk"]; wv = aps["wv"]
    bvb = aps["bvb"]; wo = aps["wo"]; maskT = aps["maskT"]; outp = aps["outp"]

    with tc.tile_pool(name="const", bufs=1) as cpool:
        bqk_sb = cpool.tile([P, 8], f32)
        bvb_sb = cpool.tile([P, CQ], f32)
        maskT_sb = cpool.tile([P, P], bf)
        ones1_sb = cpool.tile([1, 64], bf)
        xT_sb = cpool.tile([P, 8 * T], bf)
        wqk_sb = cpool.tile([P, 8 * 1024], bf)
        wv_sb = cpool.tile([P, 8 * CQ], bf)
        wo_sb = cpool.tile([P, 4 * 1024], bf)
        qkT_sb = cpool.tile([P, 8 * T], bf)
        v_sb = cpool.tile([P, 16 * VW], bf)
        attn_sb = cpool.tile([P, 16 * 512], bf)

        # DMAs in consumption order: the wqk/xT quarters the prologue
        # matmuls stream first (in cc order), small constants next, then
        # wv, the later xT quarters, and wo (only needed by stage 3).
        for cc in range(8):
            nc.sync.dma_start(wqk_sb[:, cc * 1024:(cc + 1) * 1024],
                              wqk[cc * P:(cc + 1) * P, :])
            nc.sync.dma_start(xT_sb[:, cc * T: cc * T + 512],
                              xT[cc * P:(cc + 1) * P, 0:512])
        nc.sync.dma_start(bqk_sb[:], bqk[:])
        nc.sync.dma_start(bvb_sb[:], bvb[:])
        nc.sync.dma_start(maskT_sb[:], maskT[:])
        for cc in range(8):
            nc.sync.dma_start(wv_sb[:, cc * CQ:(cc + 1) * CQ],
                              wv[cc * P:(cc + 1) * P, :])
        for cc in range(8):
            nc.sync.dma_start(xT_sb[:, cc * T + 512: cc * T + 1024],
                              xT[cc * P:(cc + 1) * P, 512:1024])
        for cc in range(8):
            nc.sync.dma_start(xT_sb[:, cc * T + 1024: (cc + 1) * T],
                              xT[cc * P:(cc + 1) * P, 1024:T])
        for hc in range(4):
            nc.sync.dma_start(wo_sb[:, hc * 1024:(hc + 1) * 1024],
                              wo[hc * P:(hc + 1) * P, :])

        nc.vector.memset(ones1_sb[:], 1.0)
        v_ones = v_sb.rearrange("p (a c) -> p a c", c=65)
        nc.vector.memset(v_ones[:, :, 64:65], 1.0)

        with tc.tile_pool(name="ps_s", bufs=1, space="PSUM") as ps_s, \
             tc.tile_pool(name="ps_av", bufs=1, space="PSUM") as ps_av, \
             tc.tile_pool(name="ps_aux", bufs=1, space="PSUM") as ps_aux, \
             tc.tile_pool(name="work", bufs=1) as wpool:

            pts = {}
            sums = {}

            # ---------- pipeline building blocks ----------
            def proj_qk(nt, tt):
                # qkT feature tile nt (q: nt=hp, k: nt=4+hp), 512 tokens
                psq = ps_aux.tile([P, 512], f32, tag="aux", bufs=2,
                                  name=f"psq_{nt}_{tt}")
                for cc in range(8):
                    nc.tensor.matmul(
                        psq[:],
                        wqk_sb[:, cc * 1024 + nt * P: cc * 1024 + (nt + 1) * P],
                        xT_sb[:, cc * T + tt * 512: cc * T + (tt + 1) * 512],
                        start=(cc == 0), stop=(cc == 7),
                    )
                nc.vector.tensor_scalar(
                    qkT_sb[:, nt * T + tt * 512: nt * T + (tt + 1) * 512],
                    psq[:], bqk_sb[:, nt:nt + 1], None, op0=AluOpType.add,
                )

            def proj_v(t16):
                psv = ps_aux.tile([P, CQ], f32, tag="aux", bufs=2,
                                  name=f"psv_{t16}")
                for cc in range(8):
                    nc.tensor.matmul(
                        psv[:],
                        xT_sb[:, cc * T + t16 * P: cc * T + (t16 + 1) * P],
                        wv_sb[:, cc * CQ:(cc + 1) * CQ],
                        start=(cc == 0), stop=(cc == 7),
                    )
                vv = v_sb[:, t16 * VW:(t16 + 1) * VW].rearrange(
                    "p (a c) -> p a c", c=65)
                nc.vector.tensor_tensor(
                    vv[:, :, 0:64],
                    psv[:].rearrange("p (a c) -> p a c", c=64),
                    bvb_sb[:].rearrange("p (a c) -> p a c", c=64),
                    op=AluOpType.add,
                )

            def s_chunk(hp, tb, j):
                h0, h1 = 2 * hp, 2 * hp + 1
                off = j * P - tb * 512
                nstart = max(off, 0)
                pss = ps_s.tile([P, 1024], f32, tag="pss", bufs=2,
                                name=f"pss_{hp}_{tb}_{j}")
                pt = wpool.tile([P, 1024], bf, tag="pt", bufs=18,
                                name=f"pt_{hp}_{tb}_{j}")
                pts[(hp, tb, j)] = pt
                for i, hl in enumerate((h0, h1)):
                    base = (hl % 2) * 64
                    nc.tensor.matmul(
                        pss[:, i * 512 + nstart: i * 512 + 512],
                        qkT_sb[base:base + 64,
                               (4 + hp) * T + j * P: (4 + hp) * T + (j + 1) * P],
                        qkT_sb[base:base + 64,
                               hp * T + tb * 512 + nstart: hp * T + (tb + 1) * 512],
                        start=True, stop=True,
                    )
                pw = pss.rearrange("p (a c) -> p a c", c=512)
                ptw = pt.rearrange("p (a c) -> p a c", c=512)
                nc.scalar.activation(
                    ptw[:, :, nstart:512], pw[:, :, nstart:512], Exp, scale=0.125,
                )
                if off >= 0:
                    # mask both heads' diagonal blocks on the otherwise-idle
                    # Pool engine (keeps the exp->mask->AV chain off the DVE
                    # queue, which is busy with AV drains at segment starts)
                    blk = pt.rearrange(
                        "p (a c) -> p a c", c=512)[:, :, nstart:nstart + P]
                    mb = maskT_sb.rearrange(
                        "p (a f) -> p a f", a=1).broadcast_to([P, 2, P])
                    nc.gpsimd.tensor_tensor(blk, blk, mb, op=AluOpType.mult)

            def av_chunk(hp, tb, j, psav):
                h0, h1 = 2 * hp, 2 * hp + 1
                jmax = 4 * tb + 3
                off = j * P - tb * 512
                nstart = max(off, 0)
                for i, hl in enumerate((h0, h1)):
                    nc.tensor.matmul(
                        psav[i][0:65, nstart:512],
                        v_sb[:, j * VW + hl * 65: j * VW + (hl + 1) * 65],
                        pts[(hp, tb, j)][:, i * 512 + nstart: i * 512 + 512],
                        start=(j == 0), stop=(j == jmax),
                    )
                if j == jmax:
                    seg = hp * NTT + tb
                    sl = slice(seg * 512, (seg + 1) * 512)
                    s_e = wpool.tile([1, 512], f32, tag="sums_e", bufs=4,
                                     name=f"sums_e_{seg}")
                    s_o = wpool.tile([1, 512], f32, tag="sums_o", bufs=4,
                                     name=f"sums_o_{seg}")
                    sums[seg] = (s_e, s_o)
                    # even head drains on DVE, odd head on ACT (concurrent),
                    # denominator rows on DVE
                    nc.vector.tensor_copy(attn_sb[0:64, sl], psav[0][0:64, :])
                    nc.scalar.copy(attn_sb[64:128, sl], psav[1][0:64, :])
                    nc.vector.tensor_copy(s_e[:], psav[0][64:65, :])
                    nc.vector.tensor_copy(s_o[:], psav[1][64:65, :])

            def norm_seg(seg):
                sl = slice(seg * 512, (seg + 1) * 512)
                s_e, s_o = sums.pop(seg)
                rec_e = wpool.tile([1, 512], bf, tag="rece", bufs=3,
                                   name=f"rece_{seg}")
                rec_o = wpool.tile([1, 512], bf, tag="reco", bufs=3,
                                   name=f"reco_{seg}")
                with nc.allow_low_precision(reason="bf16 softmax denominators"):
                    nc.vector.reciprocal(rec_e[:], s_e[:])
                    nc.vector.reciprocal(rec_o[:], s_o[:])
                psr = ps_aux.tile([P, 512], f32, tag="aux", bufs=2,
                                  name=f"psr_{seg}")
                nc.tensor.matmul(psr[0:64, :], ones1_sb[:], rec_e[:],
                                 start=True, stop=True)
                nc.tensor.matmul(psr[64:128, :], ones1_sb[:], rec_o[:],
                                 start=True, stop=True)
                nc.vector.tensor_tensor(attn_sb[:, sl], attn_sb[:, sl], psr[:],
                                        op=AluOpType.mult)

            def oproj_tt(tt16):
                psos = [
                    ps_aux.tile([P, 512], f32, tag="aux", bufs=2,
                                name=f"pso_{tt16}_{mb}")
                    for mb in range(2)
                ]
                for hc in range(4):
                    seg = hc * NTT + tt16 // 4
                    col = (seg * 4 + tt16 % 4) * P
                    for mb in range(2):
                        nc.tensor.matmul(
                            psos[mb][:],
                            attn_sb[:, col: col + P],
                            wo_sb[:, hc * 1024 + mb * 512: hc * 1024 + (mb + 1) * 512],
                            start=(hc == 0), stop=(hc == 3),
                        )
                for mb in range(2):
                    osb = wpool.tile([P, 512], bf, tag="osb", bufs=4,
                                     name=f"osb_{tt16}_{mb}")
                    nc.vector.tensor_copy(osb[:], psos[mb][:])
                    nc.sync.dma_start(
                        outp[tt16 * P:(tt16 + 1) * P, mb * 512:(mb + 1) * 512],
                        osb[:],
                    )

            # ---------- per-segment filler schedules ----------
            # Filler emitted during segment s must only feed emissions of
            # segment s+1 or later (PE queue is in-order; anything a queued
            # instruction waits on must already be in the queue).
            def fillers_for(s):
                hp, tb = s // 4, s % 4
                out = []
                if hp == 0:
                    if tb < 2:
                        out.append(lambda tt=tb + 2: proj_qk(0, tt))
                        out.append(lambda tt=tb + 2: proj_qk(4, tt))
                    if tb < 3:
                        for i in range(4):
                            out.append(lambda t=4 * (tb + 1) + i: proj_v(t))
                if hp < 3:
                    out.append(lambda h=hp + 1, tt=tb: proj_qk(h, tt))
                    out.append(lambda h=hp + 1, tt=tb: proj_qk(4 + h, tt))
                if hp == 3 and tb >= 1:
                    for i in range(4):
                        out.append(lambda t=4 * (tb - 1) + i: oproj_tt(t))
                return out

            # ---------- prologue ----------
            proj_qk(0, 0)   # q features of head pair 0, tokens 0:512
            proj_qk(4, 0)   # k features
            for j in range(4):
                s_chunk(0, 0, j)   # spool up ACT as early as possible
            for t16 in range(4):
                proj_v(t16)
            proj_qk(0, 1)   # q/k for query block 1 (its S chunks are
            proj_qk(4, 1)   # emitted inside segment 0's m-loop)

            # ---------- main pipeline ----------
            for s in range(16):
                hp, tb = s // 4, s % 4
                psav = [
                    ps_av.tile([P, 512], f32, tag="psav", bufs=2,
                               name=f"psav_{hp}_{tb}_{i}")
                    for i in range(2)
                ]
                js_a = list(range(4 * tb + 4))
                if s + 1 < 16:
                    nhp, ntb = (s + 1) // 4, (s + 1) % 4
                    js_s = list(range(4 * ntb + 4))
                else:
                    js_s = []
                fill = fillers_for(s)
                nsteps = max(len(js_a), len(js_s))
                # spread fillers over the m-loop, front-loaded after m=1
                for m in range(nsteps):
                    if s > 0 and m == 1:
                        norm_seg(s - 1)
                    if m < len(js_s):
                        s_chunk(nhp, ntb, js_s[m])
                    if m < len(js_a):
                        av_chunk(hp, tb, js_a[m], psav)
                    while fill and len(fill) >= (nsteps - m):
                        fill.pop(0)()
                while fill:
                    fill.pop(0)()

            # ---------- epilogue ----------
            norm_seg(15)
            for t16 in range(12, 16):
                oproj_tt(t16)


_cached = {}


def build_program(split=True, ncopies=1):
    key = ("nc", ncopies)
    if key not in _cached:
        import concourse.bass as bass
        import concourse.tile as tile

        nc = bass.Bass("TRN2", target_bir_lowering=False, debug=False)
        with tile.TileContext(nc) as tc:
            aps = _declare(nc)
            for _ in range(ncopies):
                _emit(nc, tc, aps)
        _cached[key] = nc
    if split and not _cached.get(("split", ncopies)):
        _split_waits(_cached[key])
        _cached[("split", ncopies)] = True
    return _cached[key]


def make_in_maps(x, W_qkv, b_qkv, W_o):
    x = np.asarray(x, dtype=np.float32)
    W_qkv = np.asarray(W_qkv, dtype=np.float32)
    b_qkv = np.asarray(b_qkv, dtype=np.float32)
    W_o = np.asarray(W_o, dtype=np.float32)
    maskT = np.triu(np.ones((P, P), np.float32)).astype(BF16)
    in_maps = []
    for core in range(NCORES):
        b, g = core // 2, core % 2
        qs = slice(g * CQ, (g + 1) * CQ)
        xTc = np.ascontiguousarray(x[b].T).astype(BF16)
        wq = W_qkv[:, 0:C][:, qs]
        wk = W_qkv[:, C:2 * C][:, qs]
        wvs = np.ascontiguousarray(W_qkv[:, 2 * C:3 * C][:, qs]).astype(BF16)
        wqks = np.ascontiguousarray(np.concatenate([wq, wk], axis=1)).astype(BF16)
        bq = b_qkv[0:C][qs]
        bk = b_qkv[C:2 * C][qs]
        bv = b_qkv[2 * C:3 * C][qs]
        bqk_t = np.ascontiguousarray(
            np.concatenate([bq, bk]).reshape(8, P).T
        ).astype(np.float32)
        bvb = np.ascontiguousarray(
            np.broadcast_to(bv, (P, CQ))
        ).astype(np.float32)
        wos = np.ascontiguousarray(W_o[qs, :]).astype(BF16)
        in_maps.append(
            dict(xT=xTc, wqk=wqks, bqk=bqk_t, wv=wvs, bvb=bvb, wo=wos,
                 maskT=maskT)
        )
    return in_maps


def run(x, W_qkv, b_qkv, W_o, b_o, trace=False, trace_kwargs=None):
    import time as _time

    from concourse.bass_utils import run_bass_kernel_spmd

    nc = build_program()
    in_maps = make_in_maps(x, W_qkv, b_qkv, W_o)
    last_err = None
    for attempt in range(3):
        try:
            res = run_bass_kernel_spmd(
                nc, in_maps, core_ids=list(range(NCORES)), trace=trace,
                **(trace_kwargs or {}),
            )
            break
        except Exception as e:  # transient device wedge -> retry
            last_err = e
            _time.sleep(5)
    else:
        raise last_err
    b_o = np.asarray(b_o, dtype=np.float32)
    out = np.empty((B, T, C), np.float32)
    for b in range(B):
        out[b] = (res.results[2 * b]["outp"].astype(np.float32)
                  + res.results[2 * b + 1]["outp"].astype(np.float32) + b_o)
    return out, res


def kernel(x, W_qkv, b_qkv, W_o, b_o):
    out, _ = run(x, W_qkv, b_qkv, W_o, b_o, trace=False)
    return out
